# revision 7
# baseline (speedup 1.0000x reference)
"""Causal multi-head attention block on 8 Trainium2 NeuronCores.

Problem: B=4, S=2048, D=1024, H=16 heads (d_k=64), causal softmax attention
with Q/K/V/O projections (torch Linear convention: y = x @ W.T + b).

Sharding: 2-way tensor parallel over heads x 4-way data parallel over batch.
Core c handles batch b = c // 2 and head group g = c % 2 (8 heads, 512
features). Each core computes its partial out-projection in bf16; the host
sums the two partials per batch and adds the bias constant
(bo + bv @ Wo.T — the V bias contributes a constant row because softmax rows
sum to 1).

Per-core kernel, v3 (fp8 DoubleRow projections with host residual
compensation; exact bf16 attention path):
  Q/K/V projections run as three fp8e4m3 DoubleRow series per PSUM —
    x8*W8 + xr*W8 + x8*Wr, where xr = fp8(x - fp8(x)) and Wr = fp8(W*SW -
    fp8(W*SW)) are HOST-computed residuals (the dropped xr*Wr term is
    ~0.2%^2).  12 DR pair-matmuls per [128, 512] PSUM = 3072 PE cycles vs
    4096 bf16 — full fp32-level accuracy at 1.33x speed.  Bias is added on
    the DVE copy out (result scaled by 1/SW).
  Attention (all bf16, exact): S^T blocks [k128, q] = K_h.T @ Q_h per
    (head, k-chunk) restricted to causal column ranges at 256 granularity;
    exp on ACT (scale=1/sqrt(d_k) folded in) -> bf16 pt [128, 2kc, 2h*512];
    diagonal-block masking in place via gpsimd affine_select / memset;
    PV accumulates va[kc] x pt into [65, 512] PSUM per head (va carries a
    ones column per head so row 64 is the softmax denominator).
    Normalize: DVE reciprocal + gpsimd partition_broadcast + DVE multiply
    into bf16 attnT [128, 4fc, 2048].
  Out-proj: bf16 attnT x Wo accumulated over 4 f-chunks, copied to bf16,
    DMA'd out (host sums the two TP partials per batch in fp32).
  Stage-A work for block i+1 is sprinkled between attention pairs of block
  i so no engine starves; exp has no row-max pass (scores are O(1) by
  construction, bf16 pt cannot overflow).
"""

import math

import ml_dtypes
import numpy as np

import concourse.bass as bass
import concourse.mybir as mybir
import concourse.tile as tile
from concourse import bacc
from concourse.bass_utils import run_bass_kernel_spmd

F32 = mybir.dt.float32
BF = mybir.dt.bfloat16
F8 = mybir.dt.float8e4
F8E5 = mybir.dt.float8e5
AF = mybir.ActivationFunctionType
ALU = mybir.AluOpType
DR = mybir.MatmulPerfMode.DoubleRow

N_CORES = 8
S = 2048
D = 1024
H = 16
DK = 64
HPC = 8          # heads per core
FC = HPC * DK    # features per core = 512
NF = FC // 128   # feature tiles of 128 = 4
NQ = S // 512    # q blocks of 512 = 4
SW = 32.0        # host prescale on fp8 weights
ISW = 1.0 / SW
SCALE = 1.0 / math.sqrt(DK)


def emit_kernel_body(tc, x8T, xrT, wqT, wqrT, wkT, wkrT, wvT, wvrT,
                     woT, bqd, bkd, out,
                     do_input_dma=True):
    nc = tc.nc
    with (
        tc.tile_pool(name="wqk", bufs=2) as wqk,        # wo bf16: 2 x 4KB
        tc.tile_pool(name="w8", bufs=12) as w8p,        # fp8 W+resid: 12 x 2KB
        tc.tile_pool(name="x8", bufs=16) as x8p,        # x8/xr fp8: 16 x 1KB
        tc.tile_pool(name="qk", bufs=8) as qkp,         # qt/kt bf16: 8 x 4KB
        tc.tile_pool(name="va", bufs=8) as vap,         # 8 x 1040B fp8
        tc.tile_pool(name="at", bufs=1) as atp,         # attnT 8KB fp8
        tc.tile_pool(name="pt", bufs=3) as ptp,         # 3 x 2KB fp8
        tc.tile_pool(name="ysb", bufs=3) as ysbp,       # 3 x 2KB bf16
        tc.tile_pool(name="small", bufs=4) as small,
        tc.tile_pool(name="sps", bufs=2, space="PSUM") as spsp,    # 2x2 banks
        tc.tile_pool(name="ps512", bufs=2, space="PSUM") as psp,   # 2x1 banks
        tc.tile_pool(name="pvps", bufs=2, space="PSUM") as pvp,    # 2x1 banks
    ):
        # ---- weights / biases ----
        def load(pool, shape, dtype, dram, name, tag="w8"):
            t = pool.tile(shape, dtype, tag=tag, name=name)
            if do_input_dma:
                nc.sync.dma_start(t[:], dram)
            else:
                nc.vector.memset(t[:, 0:1], 0.0)
            return t

        wq = [load(w8p, [128, 4, 512], F8, wqT[h], f"wq{h}") for h in range(2)]
        wqr = [load(w8p, [128, 4, 512], F8, wqrT[h], f"wqr{h}") for h in range(2)]
        wk = [load(w8p, [128, 4, 512], F8, wkT[h], f"wk{h}") for h in range(2)]
        wkr = [load(w8p, [128, 4, 512], F8, wkrT[h], f"wkr{h}") for h in range(2)]
        bq_sb = small.tile([128, NF], F32, tag="bias", name="bq")
        bk_sb = small.tile([128, NF], F32, tag="bias", name="bk")
        if do_input_dma:
            nc.sync.dma_start(bq_sb[:], bqd[:])
            nc.sync.dma_start(bk_sb[:], bkd[:])
        else:
            nc.vector.memset(bq_sb[:], 0.0)
            nc.vector.memset(bk_sb[:], 0.0)
        wv = [load(w8p, [128, 4, 512], F8, wvT[h], f"wv{h}") for h in range(2)]
        wvr = [load(w8p, [128, 4, 512], F8, wvrT[h], f"wvr{h}") for h in range(2)]
        wo = [load(wqk, [128, 2048], BF, woT[m], f"wo{m}", tag="wo") for m in range(2)]

        qt = [qkp.tile([128, 2048], BF, tag="qk", name=f"qt{j}") for j in range(NF)]
        kt = [qkp.tile([128, 2048], BF, tag="qk", name=f"kt{j}") for j in range(NF)]
        attnT = atp.tile([128, NF, 2048], BF, tag="at", name="attnT")
        va = [vap.tile([128, 2, HPC * 66], BF, tag="va", name=f"va{p}")
              for p in range(8)]
        for t in va:
            nc.gpsimd.memset(t[:], 1.0)

        xs = {}

        def load_x(i):
            x8, xr = [], []
            for t in range(4):
                ft = x8p.tile([128, 2, 512], F8, tag="x8", name="x8", bufs=16)
                rt = x8p.tile([128, 2, 512], F8, tag="x8", name="xr", bufs=16)
                if do_input_dma:
                    nc.sync.dma_start(ft[:], x8T[i * 4 + t])
                    nc.sync.dma_start(rt[:], xrT[i * 4 + t])
                else:
                    nc.vector.memset(ft[:, 0:1, 0:1], 0.0)
                    nc.vector.memset(rt[:, 0:1, 0:1], 0.0)
                x8.append(ft)
                xr.append(rt)
            xs[i] = (x8, xr)

        def proj_qk(i, w, wr, dst, b_sb, j):
            x8, xr = xs[i]
            ps = psp.tile([128, 512], F32, tag="ps512", name="ps")
            series = [(w, x8), (w, xr), (wr, x8)]
            for si, (ww, xx) in enumerate(series):
                for m in range(4):
                    lhsT = ww[m // 2][:, 2 * (m % 2):2 * (m % 2) + 2,
                                      j * 128:(j + 1) * 128]
                    nc.tensor.matmul(
                        ps[:], lhsT, xx[m][:], start=(si == 0 and m == 0),
                        stop=(si == 2 and m == 3), perf_mode=DR)
            nc.vector.tensor_scalar(
                dst[j][:, i * 512:(i + 1) * 512], ps[:], b_sb[:, j:j + 1],
                ISW, op0=ALU.add, op1=ALU.mult)

        def proj_v(i, t):
            x8, xr = xs[i]
            ps = psp.tile([128, 512], F32, tag="ps512", name="ps")
            series = [(x8, wv), (xr, wv), (x8, wvr)]
            for si, (xx, ww) in enumerate(series):
                for m in range(4):
                    lhsT = xx[m][:, :, t * 128:(t + 1) * 128]
                    rhs = ww[m // 2][:, 2 * (m % 2):2 * (m % 2) + 2, :]
                    nc.tensor.matmul(ps[:], lhsT, rhs,
                                     start=(si == 0 and m == 0),
                                     stop=(si == 2 and m == 3), perf_mode=DR)
            kc = 4 * i + t
            dst = va[kc // 2][:, kc % 2, :].rearrange(
                "p (h c) -> p h c", c=66)[:, :, 0:DK]
            nc.vector.tensor_scalar(
                dst, ps[:].rearrange("p (h c) -> p h c", c=DK), ISW, None,
                op0=ALU.mult)

        def stage_a_fillers(i):
            f = []
            for j in range(NF):
                f.append(lambda j=j: proj_qk(i, wq, wqr, qt, bq_sb, j))
            for j in range(NF):
                f.append(lambda j=j: proj_qk(i, wk, wkr, kt, bk_sb, j))
            for t in range(4):
                f.append(lambda t=t: proj_v(i, t))
            return f

        def attention(u, i, fillers, fstate):
            h0 = 2 * u
            npairs = 2 * (i + 1)
            pvs = []
            for hh in range(2):
                pv = pvp.tile([128, 512], F32, tag="pv", name="pv")
                pvs.append(pv)
            for p in range(npairs):
                q0 = max(0, (p - 2 * i) * 256)
                pt = ptp.tile([128, 2, 1024], BF, tag="pt", name="pt")
                for hh in range(2):
                    pr = hh * 64
                    sps = spsp.tile([128, 2, 512], F32, tag="sps", name="sps")
                    for s_ in range(2):
                        kc = 2 * p + s_
                        nc.tensor.matmul(
                            sps[:, s_, q0:512],
                            kt[u][pr:pr + 64, kc * 128:(kc + 1) * 128],
                            qt[u][pr:pr + 64, i * 512 + q0:(i + 1) * 512],
                            start=True, stop=True)
                    nc.scalar.activation(
                        pt[:, :, hh * 512 + q0:hh * 512 + 512],
                        sps[:, :, q0:512], AF.Exp, scale=SCALE)
                if p >= 2 * i:
                    blk = q0  # 0 for pair 2i, 256 for pair 2i+1
                    for s_ in range(2):
                        sub = pt[:, s_, :].rearrange("p (h q) -> p h q", h=2)
                        if s_ == 1:
                            nc.gpsimd.memset(sub[:, :, blk:blk + 128], 0.0)
                            tri = sub[:, :, blk + 128:blk + 256]
                        else:
                            tri = sub[:, :, blk:blk + 128]
                        nc.gpsimd.affine_select(
                            out=tri, in_=tri, compare_op=ALU.is_ge, fill=0.0,
                            base=0, pattern=[[0, 2], [1, 128]],
                            channel_multiplier=-1)
                for hh in range(2):
                    for s_ in range(2):
                        nc.tensor.matmul(
                            pvs[hh][0:65, q0:512],
                            va[p][:, s_, (h0 + hh) * 66:(h0 + hh) * 66 + 65],
                            pt[:, s_, hh * 512 + q0:hh * 512 + 512],
                            start=(p == 0 and s_ == 0),
                            stop=(p == npairs - 1 and s_ == 1),
                            skip_group_check=True)
                # sprinkle next block's stage-A work to keep ACT fed
                if fillers:
                    total = 8 * (i + 1)
                    done = u * npairs + p + 1
                    want = len(fillers) * done // total
                    while fstate[0] < want:
                        fillers[fstate[0]]()
                        fstate[0] += 1
            for hh in range(2):
                rec = small.tile([1, 512], F32, tag="rec", name="rec")
                nc.vector.reciprocal(rec[:], pvs[hh][64:65, :])
                bc = small.tile([64, 512], F32, tag="bc", name="bc")
                nc.gpsimd.partition_broadcast(bc[:], rec[:], channels=64)
                nc.vector.tensor_tensor(
                    attnT[hh * 64:hh * 64 + 64, u, i * 512:(i + 1) * 512],
                    pvs[hh][0:64, :], bc[:], op=ALU.mult)

        def outproj(i):
            for t in range(4 * i, 4 * i + 4):
                ysb = ysbp.tile([128, 1024], BF, tag="ysb", name="ysb")
                for oc in range(2):
                    ps = psp.tile([128, 512], F32, tag="ps512", name="ps")
                    for fc in range(NF):
                        nc.tensor.matmul(
                            ps[:], attnT[:, fc, t * 128:(t + 1) * 128],
                            wo[fc // 2][:, (fc % 2) * 1024 + oc * 512:
                                        (fc % 2) * 1024 + oc * 512 + 512],
                            start=(fc == 0), stop=(fc == NF - 1))
                    nc.vector.tensor_copy(
                        ysb[:, oc * 512:oc * 512 + 512], ps[:])
                nc.sync.dma_start(out[t * 128:(t + 1) * 128, :], ysb[:])

        # ---- main loop ----
        load_x(0)
        for f in stage_a_fillers(0):
            f()
        for i in range(NQ):
            if i < NQ - 1:
                load_x(i + 1)
                fillers = stage_a_fillers(i + 1)
            else:
                fillers = []
            fstate = [0]
            for u in range(4):
                attention(u, i, fillers, fstate)
            for k in range(fstate[0], len(fillers)):
                fillers[k]()
            outproj(i)


def declare_dram(nc):
    x8T = nc.dram_tensor("x8T", [16, 128, 2, 512], F8, kind="ExternalInput").ap()
    xrT = nc.dram_tensor("xrT", [16, 128, 2, 512], F8, kind="ExternalInput").ap()
    wqT = nc.dram_tensor("wqT", [2, 128, 4, 512], F8, kind="ExternalInput").ap()
    wqrT = nc.dram_tensor("wqrT", [2, 128, 4, 512], F8, kind="ExternalInput").ap()
    wkT = nc.dram_tensor("wkT", [2, 128, 4, 512], F8, kind="ExternalInput").ap()
    wkrT = nc.dram_tensor("wkrT", [2, 128, 4, 512], F8, kind="ExternalInput").ap()
    wvT = nc.dram_tensor("wvT", [2, 128, 4, 512], F8, kind="ExternalInput").ap()
    wvrT = nc.dram_tensor("wvrT", [2, 128, 4, 512], F8, kind="ExternalInput").ap()
    woT = nc.dram_tensor("woT", [2, 128, 2048], BF, kind="ExternalInput").ap()
    bq = nc.dram_tensor("bq", [128, NF], F32, kind="ExternalInput").ap()
    bk = nc.dram_tensor("bk", [128, NF], F32, kind="ExternalInput").ap()
    out = nc.dram_tensor("out", [S, D], BF, kind="ExternalOutput").ap()
    return (x8T, xrT, wqT, wqrT, wkT, wkrT, wvT, wvrT, woT, bq, bk, out)


def build_nc(reps=1):
    nc = bacc.Bacc(
        "TRN2", target_bir_lowering=False, debug=False, num_devices=N_CORES
    )
    aps = declare_dram(nc)
    with tile.TileContext(nc) as tc:
        if reps == 1:
            emit_kernel_body(tc, *aps)
        else:
            with tc.For_i(0, reps, 1):
                emit_kernel_body(tc, *aps)
    nc.finalize()
    return nc


F8NP = ml_dtypes.float8_e4m3fn


def _tile_x(xb):
    # [2048, 1024] -> [16, 128, 1024]: tile (i*4+t)[p, c*512+s] =
    # x[i*512+s, (2t+c)*128+p]
    return np.ascontiguousarray(
        xb.reshape(4, 512, 4, 2, 128).transpose(0, 2, 4, 3, 1).reshape(16, 128, 1024)
    )


def _tile_w(wT):
    # [1024, 512] -> [2, 128, 2048]: tile[half][p, c*512+f] =
    # wT[half*512 + c*128 + p, f]
    return np.ascontiguousarray(
        wT.reshape(2, 4, 128, 512).transpose(0, 2, 1, 3).reshape(2, 128, 2048)
    )


def _tile_wo(woTm):
    # [512, 1024] -> [2, 128, 2048]: tile[half][p, c*1024+o] =
    # woT[half*256 + c*128 + p, o]
    return np.ascontiguousarray(
        woTm.reshape(2, 2, 128, 1024).transpose(0, 2, 1, 3).reshape(2, 128, 2048)
    )


def _w8_pair(wT):
    # fp8 weight + residual tiles [2, 128, 4, 512] from [1024, 512] wT
    ws = wT * SW
    w8 = ws.astype(F8NP)
    wr = (ws - w8.astype(np.float32)).astype(F8NP)
    return (_tile_w(w8).reshape(2, 128, 4, 512),
            _tile_w(wr).reshape(2, 128, 4, 512))


def make_in_maps(x, Wq, bq, Wk, bk, Wv, bv, Wo, bo):
    in_maps = []
    for c in range(N_CORES):
        b, g = c // 2, c % 2
        sl = slice(g * FC, (g + 1) * FC)
        xt = _tile_x(x[b])
        x8 = xt.astype(F8NP)
        xr = (xt - x8.astype(np.float32)).astype(F8NP)
        wq8, wqr = _w8_pair(Wq[sl, :].T)
        wk8, wkr = _w8_pair(Wk[sl, :].T)
        wv8, wvr = _w8_pair(Wv[sl, :].T)
        in_maps.append(
            {
                "x8T": np.ascontiguousarray(x8.reshape(16, 128, 2, 512)),
                "xrT": np.ascontiguousarray(xr.reshape(16, 128, 2, 512)),
                "wqT": wq8, "wqrT": wqr,
                "wkT": wk8, "wkrT": wkr,
                "wvT": wv8, "wvrT": wvr,
                "woT": _tile_wo(Wo[:, sl].T.astype(ml_dtypes.bfloat16)),
                "bq": np.ascontiguousarray(bq[sl].reshape(NF, 128).T * SW),
                "bk": np.ascontiguousarray(bk[sl].reshape(NF, 128).T * SW),
            }
        )
    return in_maps


def assemble_output(per_core_outs, bv, Wo, bo):
    const = (bv @ Wo.T + bo).astype(np.float32)
    y = np.empty((4, S, D), np.float32)
    for b in range(4):
        y[b] = (
            per_core_outs[2 * b].astype(np.float32)
            + per_core_outs[2 * b + 1].astype(np.float32)
            + const
        )
    return y


def kernel(**inputs):
    inputs = {k: np.asarray(v, dtype=np.float32) for k, v in inputs.items()}
    nc = build_nc(reps=1)
    in_maps = make_in_maps(
        inputs["x"], inputs["Wq"], inputs["bq"], inputs["Wk"], inputs["bk"],
        inputs["Wv"], inputs["bv"], inputs["Wo"], inputs["bo"],
    )
    res = run_bass_kernel_spmd(nc, in_maps, core_ids=list(range(N_CORES)))
    outs = [res.results[c]["out"] for c in range(N_CORES)]
    return assemble_output(outs, inputs["bv"], inputs["Wo"], inputs["bo"])


# revision 8
# speedup vs baseline: 1.1284x; 1.1284x over previous
"""Causal multi-head attention block on 8 Trainium2 NeuronCores.

Problem: B=4, S=2048, D=1024, H=16 heads (d_k=64), causal softmax attention
with Q/K/V/O projections (torch Linear convention: y = x @ W.T + b).

Sharding: 2-way tensor parallel over heads x 4-way data parallel over batch.
Core c handles batch b = c // 2 and head group g = c % 2 (8 heads, 512
features). Each core computes its partial out-projection in bf16; the host
sums the two partials per batch and adds the bias constant
(bo + bv @ Wo.T — the V bias contributes a constant row because softmax rows
sum to 1).

Per-core kernel, v4 (all-bf16; structural improvements over baseline):
  Projections: Q/K/V accumulate [128, 512] PSUM over 8 bf16 d-chunk
    matmuls; bias added on the DVE copy to qt/kt [128, 2048] (no score
    pre-scale — 1/sqrt(d_k) is folded into exp's scale).
  Attention per (head, k-chunk pair), causal column ranges at 256
    granularity (diagonal q-blocks skip the fully-masked left half):
    S^T [k128, q] = K_h.T @ Q_h; exp on ACT -> bf16 pt [128, 2kc, 2h*512];
    diagonal-block masking IN PLACE via gpsimd affine_select / memset (no
    mask tiles, no DVE multiplies); PV accumulates va[kc] x pt into
    [65, 512] PSUM per head (va carries a ones column per head so row 64
    is the softmax denominator — no separate row-sum pass).
    Normalize: DVE reciprocal + gpsimd partition_broadcast + DVE multiply
    into bf16 attnT [128, 4fc, 2048].
  Out-proj: bf16 attnT x Wo accumulated over 4 f-chunks per s-tile,
    copied to bf16 ysb, DMA'd out per q-block (host sums the two TP
    partials per batch in fp32 and adds the bias constant).
  Stage-A work for block i+1 is sprinkled between attention pairs of
  block i so PE and ACT stay co-busy; outproj(i) runs right after block
  i's heads finish, spreading output DMA through the kernel.
"""

import math

import ml_dtypes
import numpy as np

import concourse.bass as bass
import concourse.mybir as mybir
import concourse.tile as tile
from concourse import bacc
from concourse.bass_utils import run_bass_kernel_spmd

F32 = mybir.dt.float32
BF = mybir.dt.bfloat16
F8 = mybir.dt.float8e4
F8E5 = mybir.dt.float8e5
AF = mybir.ActivationFunctionType
ALU = mybir.AluOpType
DR = mybir.MatmulPerfMode.DoubleRow

N_CORES = 8
S = 2048
D = 1024
H = 16
DK = 64
HPC = 8          # heads per core
FC = HPC * DK    # features per core = 512
NF = FC // 128   # feature tiles of 128 = 4
NQ = S // 512    # q blocks of 512 = 4
SW = 32.0        # host prescale on fp8 weights
ISW = 1.0 / SW
SCALE = 1.0 / math.sqrt(DK)


def emit_kernel_body(tc, xT, wqT, wkT, wvT, woT, bqd, bkd, out,
                     do_input_dma=True):
    nc = tc.nc
    with (
        tc.tile_pool(name="wqk", bufs=8) as wqk,        # weights bf16: 8 x 4KB
        tc.tile_pool(name="xb", bufs=8) as xbp,         # x bf16: 8 x 2KB
        tc.tile_pool(name="qk", bufs=8) as qkp,         # qt/kt bf16: 8 x 4KB
        tc.tile_pool(name="va", bufs=8) as vap,         # 8 x 1040B fp8
        tc.tile_pool(name="at", bufs=1) as atp,         # attnT 8KB fp8
        tc.tile_pool(name="pt", bufs=3) as ptp,         # 3 x 2KB fp8
        tc.tile_pool(name="ysb", bufs=3) as ysbp,       # 3 x 2KB bf16
        tc.tile_pool(name="small", bufs=4) as small,
        tc.tile_pool(name="sps", bufs=2, space="PSUM") as spsp,    # 2x2 banks
        tc.tile_pool(name="ps512", bufs=2, space="PSUM") as psp,   # 2x1 banks
        tc.tile_pool(name="pvps", bufs=2, space="PSUM") as pvp,    # 2x1 banks
    ):
        # ---- weights / biases ----
        def load(pool, shape, dtype, dram, name, tag="w"):
            t = pool.tile(shape, dtype, tag=tag, name=name)
            if do_input_dma:
                nc.sync.dma_start(t[:], dram)
            else:
                nc.vector.memset(t[:, 0:1], 0.0)
            return t

        wq = [load(wqk, [128, 2048], BF, wqT[h], f"wq{h}") for h in range(2)]
        wk = [load(wqk, [128, 2048], BF, wkT[h], f"wk{h}") for h in range(2)]
        bq_sb = small.tile([128, NF], F32, tag="bias", name="bq")
        bk_sb = small.tile([128, NF], F32, tag="bias", name="bk")
        if do_input_dma:
            nc.sync.dma_start(bq_sb[:], bqd[:])
            nc.sync.dma_start(bk_sb[:], bkd[:])
        else:
            nc.vector.memset(bq_sb[:], 0.0)
            nc.vector.memset(bk_sb[:], 0.0)
        wv = [load(wqk, [128, 2048], BF, wvT[h], f"wv{h}") for h in range(2)]
        wo = [load(wqk, [128, 2048], BF, woT[m], f"wo{m}") for m in range(2)]

        qt = [qkp.tile([128, 2048], BF, tag="qk", name=f"qt{j}") for j in range(NF)]
        kt = [qkp.tile([128, 2048], BF, tag="qk", name=f"kt{j}") for j in range(NF)]
        attnT = atp.tile([128, NF, 2048], BF, tag="at", name="attnT")
        va = [vap.tile([128, 2, HPC * 66], BF, tag="va", name=f"va{p}")
              for p in range(8)]
        for t in va:
            nc.gpsimd.memset(t[:], 1.0)

        xs = {}

        def load_x(i):
            xb = []
            for t in range(4):
                bt = xbp.tile([128, 1024], BF, tag="xb", name="xb", bufs=8)
                if do_input_dma:
                    nc.sync.dma_start(bt[:], xT[i * 4 + t])
                else:
                    nc.vector.memset(bt[:, 0:1], 0.0)
                xb.append(bt)
            xs[i] = xb

        def proj_qk(i, w, dst, b_sb, j):
            xb = xs[i]
            ps = psp.tile([128, 512], F32, tag="ps512", name="ps")
            for dc in range(8):
                lhsT = w[dc // 4][:, (dc % 4) * 512 + j * 128:
                                  (dc % 4) * 512 + (j + 1) * 128]
                rhs = xb[dc // 2][:, (dc % 2) * 512:(dc % 2) * 512 + 512]
                nc.tensor.matmul(ps[:], lhsT, rhs, start=(dc == 0),
                                 stop=(dc == 7))
            nc.vector.tensor_scalar(
                dst[j][:, i * 512:(i + 1) * 512], ps[:], b_sb[:, j:j + 1],
                None, op0=ALU.add)

        def proj_v(i, t):
            xb = xs[i]
            ps = psp.tile([128, 512], F32, tag="ps512", name="ps")
            for dc in range(8):
                lhsT = xb[dc // 2][:, (dc % 2) * 512 + t * 128:
                                   (dc % 2) * 512 + t * 128 + 128]
                rhs = wv[dc // 4][:, (dc % 4) * 512:(dc % 4) * 512 + 512]
                nc.tensor.matmul(ps[:], lhsT, rhs, start=(dc == 0),
                                 stop=(dc == 7))
            kc = 4 * i + t
            dst = va[kc // 2][:, kc % 2, :].rearrange(
                "p (h c) -> p h c", c=66)[:, :, 0:DK]
            nc.vector.tensor_copy(
                dst, ps[:].rearrange("p (h c) -> p h c", c=DK))

        def stage_a_fillers(i):
            f = []
            for j in range(NF):
                f.append(lambda j=j: proj_qk(i, wq, qt, bq_sb, j))
            for j in range(NF):
                f.append(lambda j=j: proj_qk(i, wk, kt, bk_sb, j))
            for t in range(4):
                f.append(lambda t=t: proj_v(i, t))
            return f

        def attention(u, i, fillers, fstate):
            h0 = 2 * u
            npairs = 2 * (i + 1)
            pvs = []
            for hh in range(2):
                pv = pvp.tile([128, 512], F32, tag="pv", name="pv")
                pvs.append(pv)
            for p in range(npairs):
                q0 = max(0, (p - 2 * i) * 256)
                pt = ptp.tile([128, 2, 1024], BF, tag="pt", name="pt")
                for hh in range(2):
                    pr = hh * 64
                    sps = spsp.tile([128, 2, 512], F32, tag="sps", name="sps")
                    for s_ in range(2):
                        kc = 2 * p + s_
                        nc.tensor.matmul(
                            sps[:, s_, q0:512],
                            kt[u][pr:pr + 64, kc * 128:(kc + 1) * 128],
                            qt[u][pr:pr + 64, i * 512 + q0:(i + 1) * 512],
                            start=True, stop=True)
                    nc.scalar.activation(
                        pt[:, :, hh * 512 + q0:hh * 512 + 512],
                        sps[:, :, q0:512], AF.Exp, scale=SCALE)
                if p >= 2 * i:
                    blk = q0  # 0 for pair 2i, 256 for pair 2i+1
                    for s_ in range(2):
                        sub = pt[:, s_, :].rearrange("p (h q) -> p h q", h=2)
                        if s_ == 1:
                            nc.gpsimd.memset(sub[:, :, blk:blk + 128], 0.0)
                            tri = sub[:, :, blk + 128:blk + 256]
                        else:
                            tri = sub[:, :, blk:blk + 128]
                        nc.gpsimd.affine_select(
                            out=tri, in_=tri, compare_op=ALU.is_ge, fill=0.0,
                            base=0, pattern=[[0, 2], [1, 128]],
                            channel_multiplier=-1)
                for hh in range(2):
                    for s_ in range(2):
                        nc.tensor.matmul(
                            pvs[hh][0:65, q0:512],
                            va[p][:, s_, (h0 + hh) * 66:(h0 + hh) * 66 + 65],
                            pt[:, s_, hh * 512 + q0:hh * 512 + 512],
                            start=(p == 0 and s_ == 0),
                            stop=(p == npairs - 1 and s_ == 1),
                            skip_group_check=True)
                # sprinkle next block's stage-A work to keep ACT fed
                if fillers:
                    total = 8 * (i + 1)
                    done = u * npairs + p + 1
                    want = len(fillers) * done // total
                    while fstate[0] < want:
                        fillers[fstate[0]]()
                        fstate[0] += 1
            for hh in range(2):
                rec = small.tile([1, 512], F32, tag="rec", name="rec")
                nc.vector.reciprocal(rec[:], pvs[hh][64:65, :])
                bc = small.tile([64, 512], F32, tag="bc", name="bc")
                nc.gpsimd.partition_broadcast(bc[:], rec[:], channels=64)
                nc.vector.tensor_tensor(
                    attnT[hh * 64:hh * 64 + 64, u, i * 512:(i + 1) * 512],
                    pvs[hh][0:64, :], bc[:], op=ALU.mult)

        def outproj(i):
            for t in range(4 * i, 4 * i + 4):
                ysb = ysbp.tile([128, 1024], BF, tag="ysb", name="ysb")
                for oc in range(2):
                    ps = psp.tile([128, 512], F32, tag="ps512", name="ps")
                    for fc in range(NF):
                        nc.tensor.matmul(
                            ps[:], attnT[:, fc, t * 128:(t + 1) * 128],
                            wo[fc // 2][:, (fc % 2) * 1024 + oc * 512:
                                        (fc % 2) * 1024 + oc * 512 + 512],
                            start=(fc == 0), stop=(fc == NF - 1))
                    nc.vector.tensor_copy(
                        ysb[:, oc * 512:oc * 512 + 512], ps[:])
                nc.sync.dma_start(out[t * 128:(t + 1) * 128, :], ysb[:])

        # ---- main loop ----
        load_x(0)
        for f in stage_a_fillers(0):
            f()
        for i in range(NQ):
            if i < NQ - 1:
                load_x(i + 1)
                fillers = stage_a_fillers(i + 1)
            else:
                fillers = []
            fstate = [0]
            for u in range(4):
                attention(u, i, fillers, fstate)
            for k in range(fstate[0], len(fillers)):
                fillers[k]()
            outproj(i)


def declare_dram(nc):
    xT = nc.dram_tensor("xT", [16, 128, 1024], BF, kind="ExternalInput").ap()
    wqT = nc.dram_tensor("wqT", [2, 128, 2048], BF, kind="ExternalInput").ap()
    wkT = nc.dram_tensor("wkT", [2, 128, 2048], BF, kind="ExternalInput").ap()
    wvT = nc.dram_tensor("wvT", [2, 128, 2048], BF, kind="ExternalInput").ap()
    woT = nc.dram_tensor("woT", [2, 128, 2048], BF, kind="ExternalInput").ap()
    bq = nc.dram_tensor("bq", [128, NF], F32, kind="ExternalInput").ap()
    bk = nc.dram_tensor("bk", [128, NF], F32, kind="ExternalInput").ap()
    out = nc.dram_tensor("out", [S, D], BF, kind="ExternalOutput").ap()
    return (xT, wqT, wkT, wvT, woT, bq, bk, out)


def build_nc(reps=1):
    nc = bacc.Bacc(
        "TRN2", target_bir_lowering=False, debug=False, num_devices=N_CORES
    )
    aps = declare_dram(nc)
    with tile.TileContext(nc) as tc:
        if reps == 1:
            emit_kernel_body(tc, *aps)
        else:
            with tc.For_i(0, reps, 1):
                emit_kernel_body(tc, *aps)
    nc.finalize()
    return nc


F8NP = ml_dtypes.float8_e4m3fn


def _tile_x(xb):
    # [2048, 1024] -> [16, 128, 1024]: tile (i*4+t)[p, c*512+s] =
    # x[i*512+s, (2t+c)*128+p]
    return np.ascontiguousarray(
        xb.reshape(4, 512, 4, 2, 128).transpose(0, 2, 4, 3, 1).reshape(16, 128, 1024)
    )


def _tile_w(wT):
    # [1024, 512] -> [2, 128, 2048]: tile[half][p, c*512+f] =
    # wT[half*512 + c*128 + p, f]
    return np.ascontiguousarray(
        wT.reshape(2, 4, 128, 512).transpose(0, 2, 1, 3).reshape(2, 128, 2048)
    )


def _tile_wo(woTm):
    # [512, 1024] -> [2, 128, 2048]: tile[half][p, c*1024+o] =
    # woT[half*256 + c*128 + p, o]
    return np.ascontiguousarray(
        woTm.reshape(2, 2, 128, 1024).transpose(0, 2, 1, 3).reshape(2, 128, 2048)
    )


def make_in_maps(x, Wq, bq, Wk, bk, Wv, bv, Wo, bo):
    in_maps = []
    for c in range(N_CORES):
        b, g = c // 2, c % 2
        sl = slice(g * FC, (g + 1) * FC)
        in_maps.append(
            {
                "xT": _tile_x(x[b].astype(ml_dtypes.bfloat16)),
                "wqT": _tile_w(Wq[sl, :].T.astype(ml_dtypes.bfloat16)),
                "wkT": _tile_w(Wk[sl, :].T.astype(ml_dtypes.bfloat16)),
                "wvT": _tile_w(Wv[sl, :].T.astype(ml_dtypes.bfloat16)),
                "woT": _tile_wo(Wo[:, sl].T.astype(ml_dtypes.bfloat16)),
                "bq": np.ascontiguousarray(bq[sl].reshape(NF, 128).T),
                "bk": np.ascontiguousarray(bk[sl].reshape(NF, 128).T),
            }
        )
    return in_maps


def assemble_output(per_core_outs, bv, Wo, bo):
    const = (bv @ Wo.T + bo).astype(np.float32)
    y = np.empty((4, S, D), np.float32)
    for b in range(4):
        y[b] = (
            per_core_outs[2 * b].astype(np.float32)
            + per_core_outs[2 * b + 1].astype(np.float32)
            + const
        )
    return y


def kernel(**inputs):
    inputs = {k: np.asarray(v, dtype=np.float32) for k, v in inputs.items()}
    nc = build_nc(reps=1)
    in_maps = make_in_maps(
        inputs["x"], inputs["Wq"], inputs["bq"], inputs["Wk"], inputs["bk"],
        inputs["Wv"], inputs["bv"], inputs["Wo"], inputs["bo"],
    )
    res = run_bass_kernel_spmd(nc, in_maps, core_ids=list(range(N_CORES)))
    outs = [res.results[c]["out"] for c in range(N_CORES)]
    return assemble_output(outs, inputs["bv"], inputs["Wo"], inputs["bo"])
